# revision 22
# baseline (speedup 1.0000x reference)
"""Multi-head attention (B=2, S=2048, D=1024, H=16) on 8 Trainium2 NeuronCores.

Sharding: batch x query-block, with feature-sharded K/V projections
exchanged by AllGather. Core c handles batch b = c//4 and QUERIES
[512*(c%4), 512*(c%4)+512) for ALL 16 heads, so the final output rows are
disjoint per core and no end-of-kernel reduction exists (the old
head-sharded layout paid ~50us of ReduceScatter tail driven by cross-core
launch skew). Core c also projects K/V feature slice [256*(c%4), +256)
over all tokens; the slices are AllGathered (2 token-halves per tensor,
pipelined) while the core projects Q and runs attention on the
early-arriving head pairs.

Per-core dataflow (matmul operands fp16, fp32 PSUM accumulation):
  - x^T tiles via hardware DMA-transpose, all DMA on one HWDGE queue
    (cross-queue transpose/transpose corrupts via the xbar; transpose/copy
    ping-pong costs ~8us per mode transition).
  - K^T compact feature-major -> DRAM dump -> AllGather -> per-pair
    zero-padded kz ring tiles (full-128 contraction scores, plain PE mode).
  - V token-major -> dump -> AllGather -> per-pair v ring tiles.
  - Per head pair (8 pairs): scores^T = K_h @ Q^T (plain), exp on ScalarE
    (scale folded), col-packed attn@V + ones-matmul sums (two heads per PE
    pass), reciprocal_approx_fast + multiply normalize.
  - Output projection contracts the 8 pair tiles, writes out_d directly.
"""

import numpy as np

import concourse.bass as bass  # noqa: F401  (engine namespaces via nc)
import concourse.mybir as mybir
import concourse.tile as tile
from concourse import bacc
from concourse.bass import _add_dep_helper
from concourse.bass_utils import run_bass_kernel_spmd

F32 = mybir.dt.float32
F16 = mybir.dt.float16
AF = mybir.ActivationFunctionType

B, S, D = 2, 2048, 1024
H, DH = 16, 64
NCORES = 8
GPB = 4                # cores per batch group
DS = D // GPB          # 256: per-core K/V feature slice
QL = S // GPB          # 512: per-core query block
P = 128
NDT = D // P           # 8 d_model tiles
NTT = S // P           # 16 token tiles
QCH = 512              # q width (PSUM bank = 512 fp32)
NKT = S // P           # 16 k tiles
NPAIR = H // 2         # 8 head pairs
SH = S // 2            # token half for the pipelined AllGathers
SCALE = float(1.0 / np.sqrt(DH))

REPLICA_GROUPS = [[0, 1, 2, 3], [4, 5, 6, 7]]

_CACHED_NC = None


def _build_module():
    nc = bacc.Bacc("TRN2", target_bir_lowering=False, debug=False,
                   num_devices=NCORES)

    xq_d = nc.dram_tensor("xq", [QL, D], F16, kind="ExternalInput")
    xk_d = nc.dram_tensor("xk", [S, D], F16, kind="ExternalInput")
    xv_d = nc.dram_tensor("xv", [S, D], F16, kind="ExternalInput")
    wq_d = nc.dram_tensor("wq", [D, D], F16, kind="ExternalInput")
    wk_d = nc.dram_tensor("wk", [D, DS], F16, kind="ExternalInput")
    wv_d = nc.dram_tensor("wv", [D, DS], F16, kind="ExternalInput")
    wo_d = nc.dram_tensor("wo", [D, D], F16, kind="ExternalInput")
    bq_d = nc.dram_tensor("bq", [D, 1], F32, kind="ExternalInput")
    bk_d = nc.dram_tensor("bk", [DS, 1], F32, kind="ExternalInput")
    bv_d = nc.dram_tensor("bv", [1, DS], F32, kind="ExternalInput")
    bo_d = nc.dram_tensor("bo", [1, D], F32, kind="ExternalInput")

    out_d = nc.dram_tensor("out", [QL, D], F16, kind="ExternalOutput")
    # local K^T / V dumps and AllGather outputs, one per token half
    ktl_h = [nc.dram_tensor(f"ktl{j}", [DS, SH], F16) for j in range(2)]
    agk_h = [nc.dram_tensor(f"agk{j}", [GPB * DS, SH], F16) for j in range(2)]
    vl_h = [nc.dram_tensor(f"vl{j}", [SH, DS], F16) for j in range(2)]
    agv_h = [nc.dram_tensor(f"agv{j}", [GPB * SH, DS], F16) for j in range(2)]

    with tile.TileContext(nc) as tc:
        with (
            tc.tile_pool(name="cst", bufs=1) as cst,
            tc.tile_pool(name="xt", bufs=12) as xtp,
            tc.tile_pool(name="exp", bufs=14) as expp,
            tc.tile_pool(name="rcp", bufs=2) as rcpp,
            tc.tile_pool(name="osb", bufs=3) as osbp,
            tc.tile_pool(name="psB", bufs=3, space="PSUM") as psB,
            tc.tile_pool(name="psC", bufs=1, space="PSUM") as psC,
        ):
            # Total PE ordering: chain every matmul to its predecessor
            # (nosync = scheduling-order only); on a PE-array tiling-mode
            # change (plain <-> col-tiled) add a semaphore edge so the
            # array drains before the mode flips.
            _real_matmul = nc.tensor.matmul
            _prev_mm = {"inst": None, "mode": None}

            def mm(out, lhsT, rhs, **kw):
                mode = "col" if kw.get("tile_position") is not None else "plain"
                inst = _real_matmul(out, lhsT, rhs, **kw)
                if _prev_mm["inst"] is not None:
                    _add_dep_helper(
                        inst.ins, _prev_mm["inst"].ins,
                        sync=(mode != _prev_mm["mode"]),
                        reason="pe-mode-order")
                _prev_mm["inst"] = inst
                _prev_mm["mode"] = mode
                return inst

            # ---- constants (single DMA queue, wk/wq first) ----
            wq_t = cst.tile([P, NDT, D], F16, tag="wq")
            wk_t = cst.tile([P, NDT, DS], F16, tag="wk")
            wv_t = cst.tile([P, NDT, DS], F16, tag="wv")
            wo_t = cst.tile([P, NDT, D], F16, tag="wo")
            bq_t = cst.tile([P, NDT, 1], F32, tag="bq")
            bk_t = cst.tile([P, 2, 1], F32, tag="bk")
            bv_row = cst.tile([1, DS], F32, tag="bvr")
            bo_row = cst.tile([1, D], F32, tag="bor")

            nc.sync.dma_start(wk_t[:], wk_d.rearrange("(a p) n -> p a n", p=P))
            nc.sync.dma_start(wq_t[:], wq_d.rearrange("(a p) n -> p a n", p=P))
            nc.sync.dma_start(bk_t[:], bk_d.rearrange("(a p) o -> p a o", p=P))
            nc.sync.dma_start(bq_t[:], bq_d.rearrange("(a p) o -> p a o", p=P))
            nc.sync.dma_start(bv_row[:], bv_d[:])
            nc.sync.dma_start(bo_row[:], bo_d[:])
            nc.sync.dma_start(wv_t[:], wv_d.rearrange("(a p) n -> p a n", p=P))
            nc.sync.dma_start(wo_t[:], wo_d.rearrange("(a p) n -> p a n", p=P))

            bv_b = cst.tile([P, DS], F32, tag="bvb")
            bo_b = cst.tile([P, D], F32, tag="bob")
            nc.gpsimd.partition_broadcast(bv_b[:], bv_row[:])
            nc.gpsimd.partition_broadcast(bo_b[:], bo_row[:])

            ones_t = cst.tile([P, DH], F16, tag="ones")
            nc.vector.memset(ones_t[:], 1.0)

            # ---- resident activations ----
            qt_t = cst.tile([P, NPAIR, QL], F16, tag="qt")  # Q^T per pair
            kc_t = cst.tile([P, 2, S], F16, tag="kc")   # own K^T compact
            v_own = cst.tile([P, NTT, DS], F16, tag="vown")  # own V tok-major
            an_t = cst.tile([P, NPAIR, QL], F16, tag="an")  # attn_norm^T

            # kz/v ring slots (manually indexed; pair p uses slot p%4).
            # kz zero padding is written once -- fills only ever touch the
            # same 64-row halves, so the pad rows stay zero across reuse.
            NSLOT = 3
            kzr, vr = [], []
            for s_i in range(NSLOT):
                kzs = cst.tile([P, 2, S], F16, tag=f"kz{s_i}", name=f"kz{s_i}")
                nc.vector.memset(kzs[:], 0.0)
                kzr.append(kzs)
                vs = cst.tile([P, NTT, 2 * DH], F16, tag=f"v{s_i}",
                              name=f"v{s_i}")
                vr.append(vs)

            # ---- transposed input tiles (DMA transpose, fp16) ----
            def load_xt(x_d, width):
                tiles = []
                for dt in range(NDT):
                    t = xtp.tile([P, width], F16, tag="xt", name="xt")
                    nc.sync.dma_start(
                        t[:], x_d[:, dt * P:(dt + 1) * P], transpose=True)
                    tiles.append(t)
                return tiles

            # (queue order is emission order: xk first, then xv in halves
            # around the K dumps, xq last -- see the schedule below)

            # ---- K^T projection (feature-major, dt-interleaved trickle) ----
            def proj_k_group(tcis):
                pss = {}
                for tci in tcis:
                    ps = psB.tile([P, 2 * QCH], F32, tag="sc", name=f"ps{tci}")
                    pss[tci] = ps
                for dt in range(NDT):
                    for tci in tcis:
                        ts0 = tci * QCH
                        for dot in range(2):
                            col = slice(dot * QCH, (dot + 1) * QCH)
                            mm(
                                pss[tci][:, col],
                                wk_t[:, dt, dot * P:(dot + 1) * P],
                                xt_k[dt][:, ts0:ts0 + QCH],
                                start=(dt == 0), stop=(dt == NDT - 1),
                            )
                for tci in tcis:
                    ts0 = tci * QCH
                    for dot in range(2):
                        nc.scalar.activation(
                            kc_t[:, dot, ts0:ts0 + QCH],
                            pss[tci][:, dot * QCH:(dot + 1) * QCH],
                            AF.Identity, bias=bk_t[:, dot, :])

            def dump_k(j):
                nc.sync.dma_start(
                    ktl_h[j].rearrange("(a p) n -> p a n", p=P),
                    kc_t[:, :, j * SH:(j + 1) * SH])
                nc.gpsimd.collective_compute(
                    "AllGather", mybir.AluOpType.bypass,
                    replica_groups=REPLICA_GROUPS,
                    ins=[ktl_h[j][:]], outs=[agk_h[j][:]])

            # ---- token-major V projection (one tile) + dumps ----
            def proj_v(tt):
                ps = psB.tile([P, DS], F32, tag="sc", name="psv")
                for dt in range(NDT):
                    mm(
                        ps[:],
                        xt_v[dt][:, tt * P:(tt + 1) * P],
                        wv_t[:, dt, :],
                        start=(dt == 0), stop=(dt == NDT - 1),
                    )
                nc.vector.tensor_add(v_own[:, tt, :], ps[:], bv_b[:, :])

            def dump_v(j):
                nc.sync.dma_start(
                    vl_h[j].rearrange("(a p) n -> p a n", p=P),
                    v_own[:, j * (NTT // 2):(j + 1) * (NTT // 2), :])
                nc.gpsimd.collective_compute(
                    "AllGather", mybir.AluOpType.bypass,
                    replica_groups=REPLICA_GROUPS,
                    ins=[vl_h[j][:]], outs=[agv_h[j][:]])

            # ---- Q^T projection (per pair/feature tile) ----
            def proj_q(ft):
                ps = psB.tile([P, QCH], F32, tag="sc", name="psq")
                for dt in range(NDT):
                    mm(
                        ps[:],
                        wq_t[:, dt, ft * P:(ft + 1) * P],
                        xt_q[dt][:],
                        start=(dt == 0), stop=(dt == NDT - 1),
                    )
                nc.scalar.activation(
                    qt_t[:, ft, :], ps[:], AF.Identity, bias=bq_t[:, ft, :])

            # ---- per-pair kz/v fills from the AllGather outputs ----
            def fill_pair(p):
                kzs = kzr[p % NSLOT]
                vs = vr[p % NSLOT]
                for j in range(2):
                    for i in range(2):
                        rows = slice(128 * p + 64 * i, 128 * p + 64 * i + 64)
                        nc.sync.dma_start(
                            kzs[64 * i:64 * i + 64, i, j * SH:(j + 1) * SH],
                            agk_h[j][rows, :])
                    base = (p // 2) * SH
                    csl = slice(128 * (p % 2), 128 * (p % 2) + 128)
                    nc.sync.dma_start(
                        vs[:, j * (NTT // 2):(j + 1) * (NTT // 2), :],
                        agv_h[j][base:base + SH, csl].rearrange(
                            "(a p) n -> p a n", p=P))

            # ---- attention phases (one head pair, full 2048 keys) ----
            def ph1(p):
                kzs = kzr[p % NSLOT]
                etiles = []
                for kp in range(NKT // 2):
                    sc0 = psB.tile([P, 2 * QCH], F32, tag="sc", name="sc0")
                    sc1 = psB.tile([P, 2 * QCH], F32, tag="sc", name="sc1")
                    for i, sc in ((0, sc0), (1, sc1)):
                        for j in range(2):
                            ks = (2 * kp + j) * P
                            col = slice(j * QCH, (j + 1) * QCH)
                            mm(
                                sc[:, col], kzs[:, i, ks:ks + P],
                                qt_t[:, p, :],
                                start=True, stop=True)
                    e0 = expp.tile([P, 2 * QCH], F16, tag="exp", name="e0")
                    e1 = expp.tile([P, 2 * QCH], F16, tag="exp", name="e1")
                    nc.scalar.activation(e0[:], sc0[:], AF.Exp, scale=SCALE)
                    nc.scalar.activation(e1[:], sc1[:], AF.Exp, scale=SCALE)
                    etiles.append((e0, e1))
                return etiles

            def ph2(p, etiles):
                vs = vr[p % NSLOT]
                acc = psC.tile([P, QCH], F32, tag="acc", name="acc")
                sm = psC.tile([P, QCH], F32, tag="sum", name="sm")
                for kt in range(NKT):
                    e0, e1 = etiles[kt // 2]
                    col = slice((kt % 2) * QCH, (kt % 2 + 1) * QCH)
                    st = (kt == 0)
                    sp = (kt == NKT - 1)
                    mm(
                        sm[0:64, :], ones_t[:], e0[:, col],
                        start=st, stop=sp,
                        tile_position=(0, 0), skip_group_check=True)
                    mm(
                        sm[64:128, :], ones_t[:], e1[:, col],
                        start=st, stop=sp,
                        tile_position=(0, 64), skip_group_check=True)
                    mm(
                        acc[0:64, :], vs[:, kt, 0:DH],
                        e0[:, col], start=st, stop=sp,
                        tile_position=(0, 0), skip_group_check=True)
                    mm(
                        acc[64:128, :], vs[:, kt, DH:2 * DH],
                        e1[:, col], start=st, stop=sp,
                        tile_position=(0, 64), skip_group_check=True)
                rc = rcpp.tile([P, QCH], F32, tag="rcp", name="rc")
                nc.vector.reciprocal_approx_fast(rc[:], sm[:])
                nc.vector.tensor_mul(an_t[:, p, :], acc[:], rc[:])

            # ---- output projection: contract the 8 pair tiles ----
            def po_tt(tt):
                po = psB.tile([P, 2 * QCH], F32, tag="sc", name="po")
                for half in range(2):
                    for p in range(NPAIR):
                        mm(
                            po[:, half * QCH:(half + 1) * QCH],
                            an_t[:, p, tt * P:(tt + 1) * P],
                            wo_t[:, p, half * QCH:(half + 1) * QCH],
                            start=(p == 0), stop=(p == NPAIR - 1))
                ob = osbp.tile([P, D], F16, tag="osb", name="ob")
                nc.vector.tensor_add(ob[:], po[:], bo_b[:])
                nc.sync.dma_start(out_d[tt * P:(tt + 1) * P, :], ob[:])

            # ---- emission schedule ----
            # DMA queue (FIFO, single queue): weights, xk, xv[0:4],
            # K dumps, xv[4:8], V dumps, xq, then per-pair fills.
            def load_xt_part(x_d, width, dts):
                tiles = []
                for dt in dts:
                    t = xtp.tile([P, width], F16, tag="xt", name="xt")
                    nc.sync.dma_start(
                        t[:], x_d[:, dt * P:(dt + 1) * P], transpose=True)
                    tiles.append(t)
                return tiles

            xt_k = load_xt(xk_d, S)
            xt_v = load_xt_part(xv_d, S, range(4))

            proj_k_group([0, 1, 2])
            dump_k(0)
            proj_k_group([3])
            dump_k(1)

            xt_v += load_xt_part(xv_d, S, range(4, NDT))

            for tt in range(NTT // 2):
                proj_v(tt)
            dump_v(0)
            for tt in range(NTT // 2, NTT):
                proj_v(tt)
            dump_v(1)

            xt_q = load_xt(xq_d, QL)
            for ft in range(NPAIR):
                proj_q(ft)

            for p in range(NPAIR):
                fill_pair(p)
                et = ph1(p)
                ph2(p, et)
            for tt in range(QL // P):
                po_tt(tt)

    nc.compile()
    return nc


def _get_nc():
    global _CACHED_NC
    if _CACHED_NC is None:
        _CACHED_NC = _build_module()
    return _CACHED_NC


def _make_in_maps(query, key, value, Wq, bq, Wk, bk, Wv, bv, Wo, bo):
    query = np.asarray(query, dtype=np.float32)
    key = np.asarray(key, dtype=np.float32)
    value = np.asarray(value, dtype=np.float32)
    Wq = np.asarray(Wq, dtype=np.float32)
    Wk = np.asarray(Wk, dtype=np.float32)
    Wv = np.asarray(Wv, dtype=np.float32)
    Wo = np.asarray(Wo, dtype=np.float32)
    bq = np.asarray(bq, dtype=np.float32)
    bk = np.asarray(bk, dtype=np.float32)
    bv = np.asarray(bv, dtype=np.float32)
    bo = np.asarray(bo, dtype=np.float32)

    in_maps = []
    for c in range(NCORES):
        b = c // GPB
        g = c % GPB
        fsl = slice(g * DS, (g + 1) * DS)
        qsl = slice(g * QL, (g + 1) * QL)
        in_maps.append({
            "xq": query[b][qsl].astype(np.float16),
            "xk": key[b].astype(np.float16),
            "xv": value[b].astype(np.float16),
            "wq": Wq.astype(np.float16),
            "wk": Wk[:, fsl].astype(np.float16),
            "wv": Wv[:, fsl].astype(np.float16),
            "wo": Wo.astype(np.float16),
            "bq": bq.reshape(D, 1).copy(),
            "bk": bk[fsl].reshape(DS, 1).copy(),
            "bv": bv[fsl].reshape(1, DS).copy(),
            "bo": bo.reshape(1, D).copy(),
        })
    return in_maps


def run(inputs, trace=False, trace_cores=None):
    """Run the SPMD kernel; returns (full_output, BassKernelResults)."""
    nc = _get_nc()
    in_maps = _make_in_maps(**inputs)
    res = run_bass_kernel_spmd(
        nc, in_maps, core_ids=list(range(NCORES)), trace=trace,
        trace_cores=trace_cores)
    out = np.empty((B, S, D), dtype=np.float32)
    for c in range(NCORES):
        b = c // GPB
        g = c % GPB
        out[b, g * QL:(g + 1) * QL, :] = \
            res.results[c]["out"].astype(np.float32)
    return out, res


def kernel(**inputs):
    out, _ = run(inputs, trace=False)
    return out


# revision 24
# speedup vs baseline: 1.0539x; 1.0539x over previous
"""Multi-head attention (B=2, S=2048, D=1024, H=16) on 8 Trainium2 NeuronCores.

Sharding: batch x query-block, with feature-sharded K/V projections
exchanged by AllGather. Core c handles batch b = c//4 and QUERIES
[512*(c%4), 512*(c%4)+512) for ALL 16 heads, so the final output rows are
disjoint per core and no end-of-kernel reduction exists (the old
head-sharded layout paid ~50us of ReduceScatter tail driven by cross-core
launch skew). Core c also projects K/V feature slice [256*(c%4), +256)
over all tokens; the slices are AllGathered (2 token-halves per tensor,
pipelined) while the core projects Q and runs attention on the
early-arriving head pairs.

Per-core dataflow (matmul operands fp16, fp32 PSUM accumulation):
  - x^T tiles via hardware DMA-transpose, all DMA on one HWDGE queue
    (cross-queue transpose/transpose corrupts via the xbar; transpose/copy
    ping-pong costs ~8us per mode transition).
  - K^T compact feature-major -> DRAM dump -> AllGather -> per-pair
    zero-padded kz ring tiles (full-128 contraction scores, plain PE mode).
  - V token-major -> dump -> AllGather -> per-pair v ring tiles.
  - Per head pair (8 pairs): scores^T = K_h @ Q^T (plain), exp on ScalarE
    (scale folded), col-packed attn@V + ones-matmul sums (two heads per PE
    pass), reciprocal_approx_fast + multiply normalize.
  - Output projection contracts the 8 pair tiles, writes out_d directly.
"""

import numpy as np

import concourse.bass as bass  # noqa: F401  (engine namespaces via nc)
import concourse.mybir as mybir
import concourse.tile as tile
from concourse import bacc
from concourse.bass import _add_dep_helper
from concourse.bass_utils import run_bass_kernel_spmd

F32 = mybir.dt.float32
F16 = mybir.dt.float16
AF = mybir.ActivationFunctionType

B, S, D = 2, 2048, 1024
H, DH = 16, 64
NCORES = 8
GPB = 4                # cores per batch group
DS = D // GPB          # 256: per-core K/V feature slice
QL = S // GPB          # 512: per-core query block
P = 128
NDT = D // P           # 8 d_model tiles
NTT = S // P           # 16 token tiles
QCH = 512              # q width (PSUM bank = 512 fp32)
NKT = S // P           # 16 k tiles
NPAIR = H // 2         # 8 head pairs
SH = S // 2            # token half for the pipelined AllGathers
SCALE = float(1.0 / np.sqrt(DH))

REPLICA_GROUPS = [[0, 1, 2, 3], [4, 5, 6, 7]]

_CACHED_NC = None


def _build_module():
    nc = bacc.Bacc("TRN2", target_bir_lowering=False, debug=False,
                   num_devices=NCORES)

    xq_d = nc.dram_tensor("xq", [QL, D], F16, kind="ExternalInput")
    xk_d = nc.dram_tensor("xk", [S, D], F16, kind="ExternalInput")
    xv_d = nc.dram_tensor("xv", [S, D], F16, kind="ExternalInput")
    wq_d = nc.dram_tensor("wq", [D, D], F16, kind="ExternalInput")
    wk_d = nc.dram_tensor("wk", [D, DS], F16, kind="ExternalInput")
    wv_d = nc.dram_tensor("wv", [D, DS], F16, kind="ExternalInput")
    wo_d = nc.dram_tensor("wo", [D, D], F16, kind="ExternalInput")
    bq_d = nc.dram_tensor("bq", [D, 1], F32, kind="ExternalInput")
    bk_d = nc.dram_tensor("bk", [DS, 1], F32, kind="ExternalInput")
    bv_d = nc.dram_tensor("bv", [1, DS], F32, kind="ExternalInput")
    bo_d = nc.dram_tensor("bo", [1, D], F32, kind="ExternalInput")

    out_d = nc.dram_tensor("out", [QL, D], F16, kind="ExternalOutput")
    # local K^T / V dumps and AllGather outputs, one per token half
    ktl_h = [nc.dram_tensor(f"ktl{j}", [DS, SH], F16) for j in range(2)]
    agk_h = [nc.dram_tensor(f"agk{j}", [GPB * DS, SH], F16) for j in range(2)]
    vl_h = [nc.dram_tensor(f"vl{j}", [SH, DS], F16) for j in range(2)]
    agv_h = [nc.dram_tensor(f"agv{j}", [GPB * SH, DS], F16) for j in range(2)]

    with tile.TileContext(nc) as tc:
        with (
            tc.tile_pool(name="cst", bufs=1) as cst,
            tc.tile_pool(name="xt", bufs=11) as xtp,
            tc.tile_pool(name="exp", bufs=16) as expp,
            tc.tile_pool(name="rcp", bufs=2) as rcpp,
            tc.tile_pool(name="osb", bufs=3) as osbp,
            tc.tile_pool(name="psB", bufs=3, space="PSUM") as psB,
            tc.tile_pool(name="psC", bufs=1, space="PSUM") as psC,
        ):
            # Total PE ordering: chain every matmul to its predecessor
            # (nosync = scheduling-order only); on a PE-array tiling-mode
            # change (plain <-> col-tiled) add a semaphore edge so the
            # array drains before the mode flips.
            _real_matmul = nc.tensor.matmul
            _prev_mm = {"inst": None, "mode": None}

            def mm(out, lhsT, rhs, **kw):
                mode = "col" if kw.get("tile_position") is not None else "plain"
                inst = _real_matmul(out, lhsT, rhs, **kw)
                if _prev_mm["inst"] is not None:
                    _add_dep_helper(
                        inst.ins, _prev_mm["inst"].ins,
                        sync=(mode != _prev_mm["mode"]),
                        reason="pe-mode-order")
                _prev_mm["inst"] = inst
                _prev_mm["mode"] = mode
                return inst

            # ---- constants (single DMA queue, wk/wq first) ----
            wq_t = cst.tile([P, NDT, D], F16, tag="wq")
            wk_t = cst.tile([P, NDT, DS], F16, tag="wk")
            wv_t = cst.tile([P, NDT, DS], F16, tag="wv")
            wo_t = cst.tile([P, NDT, D], F16, tag="wo")
            bq_t = cst.tile([P, NDT, 1], F32, tag="bq")
            bk_t = cst.tile([P, 2, 1], F32, tag="bk")
            bv_row = cst.tile([1, DS], F32, tag="bvr")
            bo_row = cst.tile([1, D], F32, tag="bor")

            # queue order: small/early weights, then ALL transposes, then
            # the big late-needed weights, then dumps/fills (which depend
            # on compute -- anything behind them in the FIFO would stall).
            nc.sync.dma_start(wk_t[:], wk_d.rearrange("(a p) n -> p a n", p=P))
            nc.sync.dma_start(bk_t[:], bk_d.rearrange("(a p) o -> p a o", p=P))
            nc.sync.dma_start(wv_t[:], wv_d.rearrange("(a p) n -> p a n", p=P))
            nc.sync.dma_start(bv_row[:], bv_d[:])

            bv_b = cst.tile([P, DS], F32, tag="bvb")
            bo_b = cst.tile([P, D], F32, tag="bob")
            nc.gpsimd.partition_broadcast(bv_b[:], bv_row[:])

            ones_t = cst.tile([P, DH], F16, tag="ones")
            nc.vector.memset(ones_t[:], 1.0)

            # ---- resident activations ----
            qt_t = cst.tile([P, NPAIR, QL], F16, tag="qt")  # Q^T per pair
            kc_t = cst.tile([P, 2, S], F16, tag="kc")   # own K^T compact
            v_own = cst.tile([P, NTT, DS], F16, tag="vown")  # own V tok-major
            an_t = cst.tile([P, NPAIR, QL], F16, tag="an")  # attn_norm^T

            # kz/v ring slots (manually indexed; pair p uses slot p%4).
            # kz zero padding is written once -- fills only ever touch the
            # same 64-row halves, so the pad rows stay zero across reuse.
            NSLOT = 3
            kzr, vr = [], []
            for s_i in range(NSLOT):
                kzs = cst.tile([P, 2, S], F16, tag=f"kz{s_i}", name=f"kz{s_i}")
                nc.vector.memset(kzs[:], 0.0)
                kzr.append(kzs)
                vs = cst.tile([P, NTT, 2 * DH], F16, tag=f"v{s_i}",
                              name=f"v{s_i}")
                vr.append(vs)

            # ---- transposed input tiles (DMA transpose, fp16) ----
            def load_xt(x_d, width):
                tiles = []
                for dt in range(NDT):
                    t = xtp.tile([P, width], F16, tag="xt", name="xt")
                    nc.sync.dma_start(
                        t[:], x_d[:, dt * P:(dt + 1) * P], transpose=True)
                    tiles.append(t)
                return tiles

            # (queue order is emission order: xk first, then xv in halves
            # around the K dumps, xq last -- see the schedule below)

            # ---- K^T projection (feature-major, dt-interleaved trickle) ----
            def proj_k_group(tcis):
                pss = {}
                for tci in tcis:
                    ps = psB.tile([P, 2 * QCH], F32, tag="sc", name=f"ps{tci}")
                    pss[tci] = ps
                for dt in range(NDT):
                    for tci in tcis:
                        ts0 = tci * QCH
                        for dot in range(2):
                            col = slice(dot * QCH, (dot + 1) * QCH)
                            mm(
                                pss[tci][:, col],
                                wk_t[:, dt, dot * P:(dot + 1) * P],
                                xt_k[dt][:, ts0:ts0 + QCH],
                                start=(dt == 0), stop=(dt == NDT - 1),
                            )
                for tci in tcis:
                    ts0 = tci * QCH
                    for dot in range(2):
                        nc.scalar.activation(
                            kc_t[:, dot, ts0:ts0 + QCH],
                            pss[tci][:, dot * QCH:(dot + 1) * QCH],
                            AF.Identity, bias=bk_t[:, dot, :])

            def dump_k(j):
                nc.sync.dma_start(
                    ktl_h[j].rearrange("(a p) n -> p a n", p=P),
                    kc_t[:, :, j * SH:(j + 1) * SH])
                nc.gpsimd.collective_compute(
                    "AllGather", mybir.AluOpType.bypass,
                    replica_groups=REPLICA_GROUPS,
                    ins=[ktl_h[j][:]], outs=[agk_h[j][:]])

            # ---- token-major V projection (one tile) + dumps ----
            def proj_v(tt):
                ps = psB.tile([P, DS], F32, tag="sc", name="psv")
                for dt in range(NDT):
                    mm(
                        ps[:],
                        xt_v[dt][:, tt * P:(tt + 1) * P],
                        wv_t[:, dt, :],
                        start=(dt == 0), stop=(dt == NDT - 1),
                    )
                nc.vector.tensor_add(v_own[:, tt, :], ps[:], bv_b[:, :])

            def dump_v(j):
                nc.sync.dma_start(
                    vl_h[j].rearrange("(a p) n -> p a n", p=P),
                    v_own[:, j * (NTT // 2):(j + 1) * (NTT // 2), :])
                nc.gpsimd.collective_compute(
                    "AllGather", mybir.AluOpType.bypass,
                    replica_groups=REPLICA_GROUPS,
                    ins=[vl_h[j][:]], outs=[agv_h[j][:]])

            # ---- Q^T projection (per pair/feature tile) ----
            def proj_q(ft):
                ps = psB.tile([P, QCH], F32, tag="sc", name="psq")
                for dt in range(NDT):
                    mm(
                        ps[:],
                        wq_t[:, dt, ft * P:(ft + 1) * P],
                        xt_q[dt][:],
                        start=(dt == 0), stop=(dt == NDT - 1),
                    )
                nc.scalar.activation(
                    qt_t[:, ft, :], ps[:], AF.Identity, bias=bq_t[:, ft, :])

            # ---- per-pair kz/v fills from the AllGather outputs ----
            def fill_pair(p):
                kzs = kzr[p % NSLOT]
                vs = vr[p % NSLOT]
                for j in range(2):
                    for i in range(2):
                        rows = slice(128 * p + 64 * i, 128 * p + 64 * i + 64)
                        nc.sync.dma_start(
                            kzs[64 * i:64 * i + 64, i, j * SH:(j + 1) * SH],
                            agk_h[j][rows, :])
                    base = (p // 2) * SH
                    csl = slice(128 * (p % 2), 128 * (p % 2) + 128)
                    nc.sync.dma_start(
                        vs[:, j * (NTT // 2):(j + 1) * (NTT // 2), :],
                        agv_h[j][base:base + SH, csl].rearrange(
                            "(a p) n -> p a n", p=P))

            # ---- attention phases (one head pair, full 2048 keys) ----
            def ph1(p):
                kzs = kzr[p % NSLOT]
                etiles = []
                for kp in range(NKT // 2):
                    sc0 = psB.tile([P, 2 * QCH], F32, tag="sc", name="sc0")
                    sc1 = psB.tile([P, 2 * QCH], F32, tag="sc", name="sc1")
                    for i, sc in ((0, sc0), (1, sc1)):
                        for j in range(2):
                            ks = (2 * kp + j) * P
                            col = slice(j * QCH, (j + 1) * QCH)
                            mm(
                                sc[:, col], kzs[:, i, ks:ks + P],
                                qt_t[:, p, :],
                                start=True, stop=True)
                    e0 = expp.tile([P, 2 * QCH], F16, tag="exp", name="e0")
                    e1 = expp.tile([P, 2 * QCH], F16, tag="exp", name="e1")
                    nc.scalar.activation(e0[:], sc0[:], AF.Exp, scale=SCALE)
                    nc.scalar.activation(e1[:], sc1[:], AF.Exp, scale=SCALE)
                    etiles.append((e0, e1))
                return etiles

            def ph2(p, etiles):
                vs = vr[p % NSLOT]
                acc = psC.tile([P, QCH], F32, tag="acc", name="acc")
                sm = psC.tile([P, QCH], F32, tag="sum", name="sm")
                for kt in range(NKT):
                    e0, e1 = etiles[kt // 2]
                    col = slice((kt % 2) * QCH, (kt % 2 + 1) * QCH)
                    st = (kt == 0)
                    sp = (kt == NKT - 1)
                    mm(
                        sm[0:64, :], ones_t[:], e0[:, col],
                        start=st, stop=sp,
                        tile_position=(0, 0), skip_group_check=True)
                    mm(
                        sm[64:128, :], ones_t[:], e1[:, col],
                        start=st, stop=sp,
                        tile_position=(0, 64), skip_group_check=True)
                    mm(
                        acc[0:64, :], vs[:, kt, 0:DH],
                        e0[:, col], start=st, stop=sp,
                        tile_position=(0, 0), skip_group_check=True)
                    mm(
                        acc[64:128, :], vs[:, kt, DH:2 * DH],
                        e1[:, col], start=st, stop=sp,
                        tile_position=(0, 64), skip_group_check=True)
                rc = rcpp.tile([P, QCH], F32, tag="rcp", name="rc")
                nc.vector.reciprocal_approx_fast(rc[:], sm[:])
                nc.vector.tensor_mul(an_t[:, p, :], acc[:], rc[:])

            # ---- output projection: contract the 8 pair tiles ----
            def po_tt(tt):
                po = psB.tile([P, 2 * QCH], F32, tag="sc", name="po")
                for half in range(2):
                    for p in range(NPAIR):
                        mm(
                            po[:, half * QCH:(half + 1) * QCH],
                            an_t[:, p, tt * P:(tt + 1) * P],
                            wo_t[:, p, half * QCH:(half + 1) * QCH],
                            start=(p == 0), stop=(p == NPAIR - 1))
                ob = osbp.tile([P, D], F16, tag="osb", name="ob")
                nc.vector.tensor_add(ob[:], po[:], bo_b[:])
                nc.sync.dma_start(out_d[tt * P:(tt + 1) * P, :], ob[:])

            # ---- emission schedule ----
            # DMA queue (FIFO, single queue): weights, xk, xv[0:4],
            # K dumps, xv[4:8], V dumps, xq, then per-pair fills.
            def load_xt_part(x_d, width, dts):
                tiles = []
                for dt in dts:
                    t = xtp.tile([P, width], F16, tag="xt", name="xt")
                    nc.sync.dma_start(
                        t[:], x_d[:, dt * P:(dt + 1) * P], transpose=True)
                    tiles.append(t)
                return tiles

            xt_k = load_xt(xk_d, S)
            xt_v = load_xt(xv_d, S)
            xt_q = load_xt(xq_d, QL)
            nc.sync.dma_start(wq_t[:], wq_d.rearrange("(a p) n -> p a n", p=P))
            nc.sync.dma_start(bq_t[:], bq_d.rearrange("(a p) o -> p a o", p=P))
            nc.sync.dma_start(wo_t[:], wo_d.rearrange("(a p) n -> p a n", p=P))
            nc.sync.dma_start(bo_row[:], bo_d[:])
            nc.gpsimd.partition_broadcast(bo_b[:], bo_row[:])

            proj_k_group([0, 1, 2])
            dump_k(0)
            proj_k_group([3])
            dump_k(1)

            for tt in range(NTT // 2):
                proj_v(tt)
            dump_v(0)
            for tt in range(NTT // 2, NTT):
                proj_v(tt)
            dump_v(1)

            for ft in range(NPAIR):
                proj_q(ft)

            for p in range(NPAIR):
                fill_pair(p)
                et = ph1(p)
                ph2(p, et)
            for tt in range(QL // P):
                po_tt(tt)

    nc.compile()
    return nc


def _get_nc():
    global _CACHED_NC
    if _CACHED_NC is None:
        _CACHED_NC = _build_module()
    return _CACHED_NC


def _make_in_maps(query, key, value, Wq, bq, Wk, bk, Wv, bv, Wo, bo):
    query = np.asarray(query, dtype=np.float32)
    key = np.asarray(key, dtype=np.float32)
    value = np.asarray(value, dtype=np.float32)
    Wq = np.asarray(Wq, dtype=np.float32)
    Wk = np.asarray(Wk, dtype=np.float32)
    Wv = np.asarray(Wv, dtype=np.float32)
    Wo = np.asarray(Wo, dtype=np.float32)
    bq = np.asarray(bq, dtype=np.float32)
    bk = np.asarray(bk, dtype=np.float32)
    bv = np.asarray(bv, dtype=np.float32)
    bo = np.asarray(bo, dtype=np.float32)

    in_maps = []
    for c in range(NCORES):
        b = c // GPB
        g = c % GPB
        fsl = slice(g * DS, (g + 1) * DS)
        qsl = slice(g * QL, (g + 1) * QL)
        in_maps.append({
            "xq": query[b][qsl].astype(np.float16),
            "xk": key[b].astype(np.float16),
            "xv": value[b].astype(np.float16),
            "wq": Wq.astype(np.float16),
            "wk": Wk[:, fsl].astype(np.float16),
            "wv": Wv[:, fsl].astype(np.float16),
            "wo": Wo.astype(np.float16),
            "bq": bq.reshape(D, 1).copy(),
            "bk": bk[fsl].reshape(DS, 1).copy(),
            "bv": bv[fsl].reshape(1, DS).copy(),
            "bo": bo.reshape(1, D).copy(),
        })
    return in_maps


def run(inputs, trace=False, trace_cores=None):
    """Run the SPMD kernel; returns (full_output, BassKernelResults)."""
    nc = _get_nc()
    in_maps = _make_in_maps(**inputs)
    res = run_bass_kernel_spmd(
        nc, in_maps, core_ids=list(range(NCORES)), trace=trace,
        trace_cores=trace_cores)
    out = np.empty((B, S, D), dtype=np.float32)
    for c in range(NCORES):
        b = c // GPB
        g = c % GPB
        out[b, g * QL:(g + 1) * QL, :] = \
            res.results[c]["out"].astype(np.float32)
    return out, res


def kernel(**inputs):
    out, _ = run(inputs, trace=False)
    return out
